# revision 53
# baseline (speedup 1.0000x reference)
"""MDGCRNCell Trainium2 kernel: 8-core SPMD Bass implementation.

Decomposition (B=64, N=2048, CIN=128, DOUT=64, D=16, CHEB_K=2):
  - A = softmax(relu(E E^T), axis=1). No rowmax subtraction (G max ~40, fp32
    exp is safe); denominator row-sums come from a ones-vector matmul fused
    into the propagation loop; 1/D is applied at PSUM evacuation.
  - G = relu(E E^T) is symmetric, so the [m-rows, n-cols] tile of exp(G) IS
    the [m, n] tile of M^T needed for propagation - no transposes.
  - Stage 1 (batch-sharded, f32r matmuls): each core propagates its 8
    samples over all nodes: xg1[(b,c), n] += X^T[m,(b,c)] . M^T[m, n].
  - AllToAll reshards xg batch-sharded -> node-sharded (fp16).
  - Stage 2 (node-sharded, fp16 matmuls): per-node einsum with per-node
    weights W[n] = sum_d e[n,d] Wg[d], computed per core for its 256 nodes
    and round-tripped through DRAM to obtain the [ki, co] layout.
  - GRU: z,r = sigmoid(gate). The candidate propagation reuses xg1's x and
    emb_dyn columns; only the z*state columns are re-propagated (2nd A2A for
    z*state, 3rd for the propagated result).
"""
import sys
sys.path.insert(0, '/opt/trn_rl_repo')
import ml_dtypes
import numpy as np

NC = 8
B, N, DIN, DOUT, D = 64, 2048, 32, 64, 16
CIN = DIN + DOUT + 2 * D  # 128
COg = 2 * DOUT            # 128
COu = DOUT                # 64

_cache = {}
_last_in_maps = None


def _build(n, b, reps=1, local=False, upto=9):
    import os
    import concourse.tile as tile
    from concourse import bacc, mybir

    f32 = mybir.dt.float32
    f32r = mybir.dt.float32r
    f16 = mybir.dt.float16
    bf16 = mybir.dt.bfloat16
    AF = mybir.ActivationFunctionType
    OP = mybir.AluOpType

    DBG = os.environ.get("KBENCH_DEBUG") == "1"
    NO_BC4 = os.environ.get("KB_NO_BC4") == "1"
    NO_GRE4 = os.environ.get("KB_NO_GRE4") == "1"
    NO_ZST = os.environ.get("KB_NO_ZST") == "1"

    bl = b // NC            # local batch (8)
    nl = n // NC            # local nodes / A2A chunk width (256)
    MT = n // 128           # m-chunks (16)
    NTile = 512 if n >= 2048 else nl   # P1/P4 n-tile
    NT = n // NTile
    DPT = NTile // nl       # A2A dest chunks per n-tile
    TB = min(32, nl)        # stage-2 node batch
    NBat = nl // TB
    BC1 = bl * CIN // 128   # d1 bc-chunks (8)
    BC2 = bl * DOUT // 128  # d2 bc-chunks (4)
    WKg = 2 * CIN * COg     # 32768
    WKu = 2 * CIN * COu     # 16384

    nc = bacc.Bacc("TRN2", target_bir_lowering=False, num_devices=NC)

    xprop_d = nc.dram_tensor("xprop", [n, bl, CIN], bf16, kind="ExternalInput")
    inpT_d = nc.dram_tensor("inpT", [CIN, nl, b], f16, kind="ExternalInput")
    statT_d = nc.dram_tensor("statT", [DOUT, nl, b], f16, kind="ExternalInput")
    eT_d = nc.dram_tensor("eT", [D, n], f32, kind="ExternalInput")
    eTown_d = nc.dram_tensor("eTown", [D, nl], f32, kind="ExternalInput")
    wgp_d = nc.dram_tensor("wgp", [D, WKg], f16, kind="ExternalInput")
    wup_d = nc.dram_tensor("wup", [D, WKu], f16, kind="ExternalInput")
    bgp_d = nc.dram_tensor("bgp", [D, COg], f16, kind="ExternalInput")
    bup_d = nc.dram_tensor("bup", [D, COu], f16, kind="ExternalInput")
    out_d = nc.dram_tensor("out", [nl, COu, b], f16, kind="ExternalOutput")
    if DBG:
        dbgDinv_d = nc.dram_tensor("dbgDinv", [128, n], f32, kind="ExternalOutput")
        dbgS1r_d = nc.dram_tensor("dbgS1r", [NC, CIN, bl, nl], f16, kind="ExternalOutput")
        dbgZr_d = nc.dram_tensor("dbgZr", [128, nl, b], f16, kind="ExternalOutput")
        dbgW_d = nc.dram_tensor("dbgW", [nl, 1024], f16, kind="ExternalOutput")
        dbgS2r_d = nc.dram_tensor("dbgS2r", [NC, DOUT, bl, nl], f16, kind="ExternalOutput")
        dbgCand_d = nc.dram_tensor("dbgCand", [CIN, nl, b], f16, kind="ExternalOutput")
        dbgZsR_d = nc.dram_tensor("dbgZsR", [NC, nl, bl, DOUT], bf16, kind="ExternalOutput")
        dbgMt4_d = nc.dram_tensor("dbgMt4", [128, MT, NTile], bf16, kind="ExternalOutput")

    with tile.TileContext(nc) as tc:
        with (
            tc.tile_pool(name="persist", bufs=1) as pp,
            tc.tile_pool(name="dram", bufs=1, space="DRAM") as dr,
        ):
            wdram_g = dr.tile([nl, WKg], f16)
            wdram_u = dr.tile([nl, WKu], f16)
            s1_send = dr.tile([NC, CIN, bl, nl], f16)
            s1_recv = dr.tile([NC, CIN, bl, nl], f16)
            zs_send = dr.tile([NC, nl, bl, DOUT], bf16)
            zs_recv = dr.tile([NC, nl, bl, DOUT], bf16)
            s2_send = dr.tile([NC, DOUT, bl, nl], f16)
            s2_recv = dr.tile([NC, DOUT, bl, nl], f16)
            mdram = dr.tile([NT, 128, MT, NTile], bf16)

            eT_sb = pp.tile([128, n], f32r)
            eTown16 = pp.tile([D, nl], f16)
            ones_r = pp.tile([128, 1], bf16)
            dinv_bc = pp.tile([128, n], f32)
            inpT_sb = pp.tile([CIN, nl, b], f16)
            zr16 = pp.tile([128, nl, b], f16)
            bias_g = pp.tile([COg, nl], f32)
            bias_u = pp.tile([128, nl], f32)   # rows 64:128

            def a2a(send, recv):
                if local:
                    nc.sync.dma_start(recv[:], send[:])
                else:
                    nc.gpsimd.collective_compute(
                        "AllToAll", OP.bypass, replica_groups=[list(range(NC))],
                        ins=[send[:].opt()], outs=[recv[:].opt()],
                    )

            for _rep in range(reps):
                # ========== P0: setup + W-compute + bias ==========
                nc.sync.dma_start(inpT_sb[:], inpT_d[:])
                with tc.tile_pool(name="p0tmp", bufs=1) as p0:
                    zscr = p0.tile([128, n], f32)
                    nc.vector.memset(zscr[:], 0.0)
                    nc.vector.tensor_copy(eT_sb[:], zscr[:])
                    nc.sync.dma_start(eT_sb[0:D, :], eT_d[:].bitcast(f32r))
                    eotmp = p0.tile([D, nl], f32)
                    nc.sync.dma_start(eotmp[:], eTown_d[:])
                    nc.vector.tensor_copy(eTown16[:], eotmp[:])
                    nc.vector.memset(ones_r[:], 1.0)

                    bg_sb = p0.tile([D, COg], f16)
                    bu_sb = p0.tile([D, COu], f16)
                    nc.sync.dma_start(bg_sb[:], bgp_d[:])
                    nc.sync.dma_start(bu_sb[:], bup_d[:])
                    with tc.tile_pool(name="psb", bufs=1, space="PSUM") as psb:
                        pb = psb.tile([COg, nl], f32)
                        nc.tensor.matmul(pb[:], bg_sb[:], eTown16[:], start=True, stop=True)
                        nc.vector.tensor_copy(bias_g[:], pb[:])
                        pbu = psb.tile([128, nl], f32, tag="pbu")
                        nc.tensor.matmul(
                            pbu[64:128, :], bu_sb[:], eTown16[:],
                            start=True, stop=True, tile_position=(0, 64),
                        )
                        nc.vector.tensor_copy(bias_u[64:128, :], pbu[64:128, :])

                if upto < 1:
                    continue
                # ========== P1: d1 propagation (+ W-compute filling gaps) ==========
                QW = WKg // 8
                with (
                    tc.tile_pool(name="wpool", bufs=2) as wp,
                    tc.tile_pool(name="wps", bufs=2, space="PSUM") as wps,
                    tc.tile_pool(name="wev", bufs=2) as wev,
                ):
                    with (
                        tc.tile_pool(name="xpool", bufs=1) as xp,
                        tc.tile_pool(name="mwork", bufs=1) as mwa,
                        tc.tile_pool(name="mwork2", bufs=2) as mw,
                        tc.tile_pool(name="evw", bufs=2) as evw,
                        tc.tile_pool(name="dsm", bufs=2) as dsm,
                        tc.tile_pool(name="pg", bufs=2, space="PSUM") as pgp,
                        tc.tile_pool(name="pbc", bufs=3, space="PSUM") as pbcp,
                        tc.tile_pool(name="pdp", bufs=1, space="PSUM") as pdp,
                    ):
                        # W-compute chunks, emitted interleaved between P1's
                        # nt iterations so engine-queue order matches
                        # readiness (avoids head-of-line blocking).
                        def emit_w_quarter(wsrc, wdst, h):
                            wg_sb = wp.tile([D, QW], f16, tag="wg_sb", bufs=2)
                            nc.sync.dma_start(wg_sb[:], wsrc[:, h * QW:(h + 1) * QW])
                            for nch in range(nl // 128):
                                for g4 in range(QW // 4096):
                                    wstg = wev.tile([128, 4096], f16, tag="wstg")
                                    for sl in range(8):
                                        pw = wps.tile([128, 512], f32)
                                        nc.tensor.matmul(
                                            pw[:],
                                            eTown16[:, nch * 128:(nch + 1) * 128],
                                            wg_sb[:, g4 * 4096 + sl * 512:
                                                  g4 * 4096 + sl * 512 + 512],
                                            start=True, stop=True,
                                        )
                                        if sl % 2 == 0:
                                            nc.vector.tensor_copy(
                                                wstg[:, sl * 512:(sl + 1) * 512], pw[:]
                                            )
                                        else:
                                            nc.scalar.copy(
                                                wstg[:, sl * 512:(sl + 1) * 512], pw[:]
                                            )
                                    nc.sync.dma_start(
                                        wdst[nch * 128:(nch + 1) * 128,
                                             h * QW + g4 * 4096: h * QW + g4 * 4096 + 4096],
                                        wstg[:],
                                    )

                        wq = ([(wgp_d, wdram_g, h) for h in range(WKg // QW)]
                              + [(wup_d, wdram_u, h) for h in range(WKu // QW)])

                        xprop_sb = xp.tile([128, MT, bl, CIN], bf16)
                        for mc in range(MT):
                            nc.sync.dma_start(
                                xprop_sb[:, mc, :, :],
                                xprop_d[mc * 128:(mc + 1) * 128, :, :],
                            )
                        for nt in range(NT):
                            nsl = slice(nt * NTile, (nt + 1) * NTile)
                            mt2all = mwa.tile([128, MT, NTile], bf16, tag="mt2", bufs=2)
                            pd = pdp.tile([1, NTile], f32)
                            for mc in range(MT):
                                pg = pgp.tile([128, NTile], f32)
                                nc.tensor.matmul(
                                    pg[:], eT_sb[:, mc * 128:(mc + 1) * 128],
                                    eT_sb[:, nsl], start=True, stop=True,
                                )
                                eb = mw.tile([128, NTile], f32, tag="eb")
                                nc.scalar.activation(eb[:], pg[:], AF.Relu)
                                nc.scalar.activation(mt2all[:, mc, :], eb[:], AF.Exp)
                                nc.tensor.matmul(
                                    pd[:], ones_r[:], mt2all[:, mc, :],
                                    start=(mc == 0), stop=(mc == MT - 1),
                                )
                            nc.sync.dma_start(mdram[nt], mt2all[:])
                            dnv = dsm.tile([1, NTile], f32, tag="dnv")
                            nc.vector.reciprocal(dnv[:], pd[:])
                            nc.gpsimd.partition_broadcast(dinv_bc[:, nsl], dnv[:])
                            ev_all = evw.tile([128, BC1, NTile], f16, tag="ev_all")
                            for p in range(BC1):
                                pbc = pbcp.tile([128, NTile], f32, tag="pbc", name=f"pbc{nt}_{p}")
                                for mc in range(MT):
                                    nc.tensor.matmul(
                                        pbc[:], xprop_sb[:, mc, p, :], mt2all[:, mc, :],
                                        start=(mc == 0), stop=(mc == MT - 1),
                                    )
                                nc.vector.tensor_tensor(
                                    ev_all[:, p, :], pbc[:], dinv_bc[:, nsl], OP.mult
                                )
                            for h in range(DPT):
                                nc.sync.dma_start(
                                    s1_send[nt * DPT + h, :, :, :],
                                    ev_all[:, :, h * nl:(h + 1) * nl],
                                )
                            # interleave one W quarter after each nt
                            if nt < len(wq):
                                emit_w_quarter(*wq[nt])
                        # remaining W quarters run in the A2A#1 window
                        for q in wq[NT:]:
                            emit_w_quarter(*q)

                if upto < 2:
                    continue
                a2a(s1_send, s1_recv)
                if DBG:
                    nc.sync.dma_start(dbgS1r_d[:], s1_recv[:])
                    nc.sync.dma_start(dbgDinv_d[:], dinv_bc[:])
                    nc.sync.dma_start(dbgW_d[:], wdram_g[:, 0:1024])

                if upto < 3:
                    continue
                # ========== P3: stage-2 gate ==========
                with (
                    tc.tile_pool(name="ldp", bufs=1) as ldp,
                    tc.tile_pool(name="s2w", bufs=2) as s2w,
                    tc.tile_pool(name="pz", bufs=2, space="PSUM") as pzp,
                ):
                    ld_all = ldp.tile([128, NC, bl, nl], f16)
                    for s in range(NC):
                        nc.sync.dma_start(ld_all[:, s, :, :], s1_recv[s, :, :, :])
                    for t in range(NBat):
                        tsl = slice(t * TB, (t + 1) * TB)
                        xgk1 = s2w.tile([128, TB, b], f16, tag="xgk1")
                        nc.vector.tensor_copy(
                            xgk1[:].rearrange("c n (s b) -> c n s b", s=NC),
                            ld_all[:, :, :, tsl].rearrange("c s b n -> c n s b"),
                        )
                        wg_t = s2w.tile([128, TB, 2, COg], f16, tag="wg_t", bufs=2)
                        nc.sync.dma_start(
                            wg_t[:],
                            wdram_g[tsl, :].rearrange("n (k i o) -> i n k o", k=2, i=CIN),
                        )
                        for g in range(TB // 8):
                            pz = pzp.tile([COg, 8, b], f32)
                            for j8 in range(8):
                                j = g * 8 + j8
                                nc.tensor.matmul(
                                    pz[:, j8, :], wg_t[:, j, 0, :],
                                    inpT_sb[:, t * TB + j, :],
                                    start=(j8 == 0), stop=False, skip_group_check=True,
                                )
                                nc.tensor.matmul(
                                    pz[:, j8, :], wg_t[:, j, 1, :], xgk1[:, j, :],
                                    start=False, stop=(j8 == 7), skip_group_check=True,
                                )
                            zrp = s2w.tile([COg, 8, b], f32, tag="zrp")
                            nc.vector.tensor_tensor(
                                zrp[:], pz[:],
                                bias_g[:, t * TB + g * 8: t * TB + g * 8 + 8, None]
                                .to_broadcast((COg, 8, b)),
                                OP.add,
                            )
                            nc.scalar.activation(
                                zr16[:, t * TB + g * 8: t * TB + g * 8 + 8, :], zrp[:],
                                AF.Sigmoid,
                            )
                        st3 = s2w.tile([DOUT, TB, b], f16, tag="st3", bufs=2)
                        nc.sync.dma_start(st3[:], statT_d[:, tsl, :])
                        zs16 = s2w.tile([DOUT, TB, b], f16, tag="zs16", bufs=2)
                        nc.vector.tensor_tensor(
                            zs16[:], zr16[0:DOUT, tsl, :], st3[:], OP.mult
                        )
                        nc.scalar.copy(inpT_sb[DIN:DIN + 32, tsl, :], zs16[0:32, :, :])
                        nc.scalar.copy(
                            inpT_sb[DIN + 32:DIN + DOUT, tsl, :], zs16[32:64, :, :]
                        )
                        zsbf = s2w.tile([DOUT, TB, b], bf16, tag="zsbf", bufs=2)
                        nc.vector.tensor_tensor(
                            zsbf[:], zr16[0:DOUT, tsl, :], st3[:], OP.mult
                        )
                        # c<->n transpose so the A2A payload is node-major
                        # (receiver loads [m-part, b, c] contiguously).
                        ztile = s2w.tile([TB, b, DOUT], bf16, tag="ztile", bufs=2)
                        for ci in range(2):
                            nc.vector.transpose(
                                ztile[:, :, 32 * ci:32 * ci + 32],
                                zsbf[32 * ci:32 * ci + 32, :, :]
                                .rearrange("c n b -> c b n"),
                            )
                        nc.sync.dma_start(
                            zs_send[:, tsl, :, :].rearrange("d n b c -> n d (b c)"),
                            ztile[:].rearrange("n (d bl) c -> n d (bl c)", d=NC),
                        )

                if upto < 4:
                    continue
                # ========== A2A #2 + d2 propagation ==========
                a2a(zs_send, zs_recv)
                if DBG:
                    nc.sync.dma_start(dbgZr_d[:], zr16[:])
                    nc.sync.dma_start(dbgCand_d[:], inpT_sb[:])
                    nc.sync.dma_start(dbgZsR_d[:], zs_recv[:])
                with (
                    tc.tile_pool(name="zpool", bufs=1) as zp,
                    tc.tile_pool(name="ev4", bufs=2) as ev4w,
                    tc.tile_pool(name="pbc2", bufs=3, space="PSUM") as pbcp2,
                ):
                    zsT_sb = zp.tile([128, MT, bl, DOUT], bf16)
                    if NO_ZST:
                        nc.vector.memset(zsT_sb[:], 0.5)
                    else:
                        for s in range(NC):
                            nc.sync.dma_start(
                                zsT_sb[:, 2 * s:2 * s + 2, :, :],
                                zs_recv[s].rearrange("(h m) b c -> m h b c", h=2),
                            )
                    for nt in range(NT):
                        nsl = slice(nt * NTile, (nt + 1) * NTile)
                        mt2all4 = zp.tile([128, MT, NTile], bf16, tag="mt2all4", bufs=2)
                        if NO_GRE4:
                            nc.vector.memset(mt2all4[:], 0.001)
                        else:
                            nc.sync.dma_start(mt2all4[:], mdram[nt])
                        if NO_BC4:
                            continue
                        for p in range(BC2):
                            pbc = pbcp2.tile([128, NTile], f32, tag="pbc2", name=f"pb2_{nt}_{p}")
                            for mc in range(MT):
                                nc.tensor.matmul(
                                    pbc[:], zsT_sb[:, mc, 2 * p:2 * p + 2, :],
                                    mt2all4[:, mc, :],
                                    start=(mc == 0), stop=(mc == MT - 1),
                                )
                            ev = ev4w.tile([128, NTile], f16, tag="ev")
                            nc.vector.tensor_tensor(
                                ev[:], pbc[:], dinv_bc[:, nsl], OP.mult
                            )
                            for h in range(DPT):
                                nc.sync.dma_start(
                                    s2_send[nt * DPT + h, :, 2 * p, :],
                                    ev[0:DOUT, h * nl:(h + 1) * nl],
                                )
                                nc.sync.dma_start(
                                    s2_send[nt * DPT + h, :, 2 * p + 1, :],
                                    ev[DOUT:128, h * nl:(h + 1) * nl],
                                )
                    if DBG:
                        nc.sync.dma_start(dbgMt4_d[:], mt2all4[:])

                if upto < 5:
                    continue
                # ========== A2A #3 + stage-2 update + GRU ==========
                a2a(s2_send, s2_recv)
                if DBG:
                    nc.sync.dma_start(dbgS2r_d[:], s2_recv[:])
                with (
                    tc.tile_pool(name="ldp5", bufs=1) as ldp5,
                    tc.tile_pool(name="s5w", bufs=2) as s5w,
                    tc.tile_pool(name="ph", bufs=2, space="PSUM") as php,
                ):
                    def load_wu(t):
                        wt = s5w.tile([128, TB, 2, COu], f16, tag="wu_t", bufs=3)
                        nc.sync.dma_start(
                            wt[:],
                            wdram_u[t * TB:(t + 1) * TB, :]
                            .rearrange("n (k i o) -> i n k o", k=2, i=CIN),
                        )
                        return wt

                    # prefetch wu weights ahead of the A2A-gated ld5 loads
                    wu_tiles = [load_wu(t) for t in range(min(3, NBat))]
                    ld5 = ldp5.tile([128, NC, bl, nl], f16)
                    for s in range(NC):
                        nc.sync.dma_start(ld5[0:DIN, s, :, :], s1_recv[s, 0:DIN, :, :])
                        nc.sync.dma_start(ld5[DIN:DIN + DOUT, s, :, :], s2_recv[s, :, :, :])
                        nc.sync.dma_start(
                            ld5[DIN + DOUT:CIN, s, :, :], s1_recv[s, DIN + DOUT:CIN, :, :]
                        )
                    for t in range(NBat):
                        tsl = slice(t * TB, (t + 1) * TB)
                        wu_t = wu_tiles[t]
                        if t + 3 < NBat:
                            wu_tiles.append(load_wu(t + 3))
                        xgk2 = s5w.tile([128, TB, b], f16, tag="xgk2")
                        nc.vector.tensor_copy(
                            xgk2[:].rearrange("c n (s b) -> c n s b", s=NC),
                            ld5[:, :, :, tsl].rearrange("c s b n -> c n s b"),
                        )
                        st5 = s5w.tile([DOUT, TB, b], f16, tag="st5")
                        nc.sync.dma_start(st5[:], statT_d[:, tsl, :])
                        hc_all = s5w.tile([DOUT, TB, b], f16, tag="hc_all")
                        for g in range(TB // 8):
                            gsl8 = slice(g * 8, g * 8 + 8)
                            ph = php.tile([128, 8, b], f32)
                            for j8 in range(8):
                                j = g * 8 + j8
                                nc.tensor.matmul(
                                    ph[64:128, j8, :], wu_t[:, j, 0, :],
                                    inpT_sb[:, t * TB + j, :],
                                    start=(j8 == 0), stop=False, tile_position=(0, 64),
                                    skip_group_check=True,
                                )
                                nc.tensor.matmul(
                                    ph[64:128, j8, :], wu_t[:, j, 1, :], xgk2[:, j, :],
                                    start=False, stop=(j8 == 7), tile_position=(0, 64),
                                    skip_group_check=True,
                                )
                            hcp = s5w.tile([COu, 8, b], f16, tag="hcp")
                            nc.vector.tensor_tensor(
                                hcp[:], ph[64:128, :, :],
                                bias_u[64:128, t * TB + g * 8: t * TB + g * 8 + 8, None]
                                .to_broadcast((COu, 8, b)),
                                OP.add,
                            )
                            nc.scalar.activation(hc_all[:, gsl8, :], hcp[:], AF.Tanh)
                        # h = hc + r * (state - hc), batched per t in f16 (4x DVE)
                        r16 = s5w.tile([DOUT, TB, b], f16, tag="r16")
                        nc.vector.tensor_copy(r16[:], zr16[64:128, tsl, :])
                        hfin = s5w.tile([DOUT, TB, b], f16, tag="hfin")
                        nc.vector.tensor_tensor(hfin[:], st5[:], hc_all[:], OP.subtract)
                        nc.vector.tensor_tensor(hfin[:], hfin[:], r16[:], OP.mult)
                        nc.vector.tensor_tensor(hfin[:], hfin[:], hc_all[:], OP.add)
                        nc.sync.dma_start(
                            out_d[tsl, :, :].rearrange("n c b -> c n b"),
                            hfin[:],
                        )

    nc.compile()
    return nc


def _get_nc(n, b, reps=1, local=False, upto=9):
    key = (n, b, reps, local, upto)
    if key not in _cache:
        _cache[key] = _build(n, b, reps, local, upto)
    return _cache[key]


def kernel(x, state, emb_dyn, emb_static, Wg, bg, Wu, bu):
    from concourse.bass_utils import run_bass_kernel_spmd

    x = np.asarray(x, np.float32)
    state = np.asarray(state, np.float32)
    emb_dyn = np.asarray(emb_dyn, np.float32)
    emb_static = np.asarray(emb_static, np.float32)
    b, n, _ = x.shape
    bl, nl = b // NC, n // NC

    nc = _get_nc(n, b)

    cat = np.concatenate([x, state, emb_dyn], axis=2)          # [b, n, CIN]
    eT = np.ascontiguousarray(emb_static.T, np.float32)        # [D, n]
    wgp = np.ascontiguousarray(np.asarray(Wg, np.float32).reshape(D, -1)).astype(np.float16)
    wup = np.ascontiguousarray(np.asarray(Wu, np.float32).reshape(D, -1)).astype(np.float16)
    bgp = np.asarray(bg, np.float32).astype(np.float16)
    bup = np.asarray(bu, np.float32).astype(np.float16)

    in_maps = []
    for c in range(NC):
        bsl = slice(c * bl, (c + 1) * bl)
        nsl = slice(c * nl, (c + 1) * nl)
        in_maps.append({
            "xprop": np.ascontiguousarray(cat[bsl].transpose(1, 0, 2)).astype(ml_dtypes.bfloat16),
            "inpT": np.ascontiguousarray(cat[:, nsl].transpose(2, 1, 0)).astype(np.float16),
            "statT": np.ascontiguousarray(state[:, nsl].transpose(2, 1, 0)).astype(np.float16),
            "eT": eT,
            "eTown": np.ascontiguousarray(emb_static[nsl].T, np.float32),
            "wgp": wgp, "wup": wup, "bgp": bgp, "bup": bup,
        })

    global _last_in_maps
    _last_in_maps = in_maps
    res = run_bass_kernel_spmd(nc, in_maps, core_ids=list(range(NC)))

    out = np.empty((b, n, DOUT), np.float32)
    for c in range(NC):
        nsl = slice(c * nl, (c + 1) * nl)
        out[:, nsl, :] = res.results[c]["out"].transpose(2, 0, 1).astype(np.float32)
    return out



# revision 60
# speedup vs baseline: 1.0209x; 1.0209x over previous
"""MDGCRNCell Trainium2 kernel: 8-core SPMD Bass implementation.

Decomposition (B=64, N=2048, CIN=128, DOUT=64, D=16, CHEB_K=2):
  - A = softmax(relu(E E^T), axis=1). No rowmax subtraction: M = exp(relu(G))
    is kept UNNORMALIZED in bf16 (bf16 has fp32's exponent range, so e^40
    fits); row-sums come from a ones-vector matmul; 1/D is applied at PSUM
    evacuation. relu+exp both run on the Scalar engine (consecutive
    same-engine ops avoid FIFO ping-pong); G stays f32 until after exp.
  - G is symmetric, so the [m-rows, n-cols] tile of exp(G) IS the [m, n]
    tile of M^T needed for propagation - no transposes. M is spilled to
    DRAM in P1 and reloaded in P4 (cheaper than recomputing EE^T+exp).
  - Stage 1 (batch-sharded, bf16 matmuls): each core propagates its 8
    samples over all nodes: xg1[(b,c), n] += X^T[m,(b,c)] . M^T[m, n].
    The per-node weight pools W[n] = sum_d e[n,d] W[d] are materialized
    to DRAM concurrently, emitted interleaved between P1's n-tiles so
    engine-queue order matches readiness (no head-of-line blocking).
  - AllToAll #1 reshards xg batch-sharded -> node-sharded (fp16).
  - Stage 2 (node-sharded, fp16 matmuls): per-node einsum reading W tiles
    back in [i, n, k, o] layout via strided DMA.
  - GRU: z,r = sigmoid(gate). The candidate pass reuses xg1's x/emb_dyn
    columns; only z*state is re-propagated: it is c<->n transposed on-chip
    (2 StreamTranspose ops/tile) so the A2A#2 payload is node-major and the
    receive side loads [m-part, b, c] contiguously (the naive transposed
    DRAM read was ~900us). A2A#3 returns the propagated candidate; the
    final GRU combine runs in f16 (4x DVE mode) with f16 output.
"""
import sys
sys.path.insert(0, '/opt/trn_rl_repo')
import ml_dtypes
import numpy as np

NC = 8
B, N, DIN, DOUT, D = 64, 2048, 32, 64, 16
CIN = DIN + DOUT + 2 * D  # 128
COg = 2 * DOUT            # 128
COu = DOUT                # 64

_cache = {}
_last_in_maps = None


def _build(n, b, reps=1, local=False, upto=9):
    import os
    import concourse.tile as tile
    from concourse import bacc, mybir

    f32 = mybir.dt.float32
    f32r = mybir.dt.float32r
    f16 = mybir.dt.float16
    bf16 = mybir.dt.bfloat16
    AF = mybir.ActivationFunctionType
    OP = mybir.AluOpType

    DBG = os.environ.get("KBENCH_DEBUG") == "1"
    NO_BC4 = os.environ.get("KB_NO_BC4") == "1"
    NO_GRE4 = os.environ.get("KB_NO_GRE4") == "1"
    NO_ZST = os.environ.get("KB_NO_ZST") == "1"

    bl = b // NC            # local batch (8)
    nl = n // NC            # local nodes / A2A chunk width (256)
    MT = n // 128           # m-chunks (16)
    NTile = 512 if n >= 2048 else nl   # P1/P4 n-tile
    NT = n // NTile
    DPT = NTile // nl       # A2A dest chunks per n-tile
    TB = min(32, nl)        # stage-2 node batch
    NBat = nl // TB
    BC1 = bl * CIN // 128   # d1 bc-chunks (8)
    BC2 = bl * DOUT // 128  # d2 bc-chunks (4)
    WKg = 2 * CIN * COg     # 32768
    WKu = 2 * CIN * COu     # 16384

    nc = bacc.Bacc("TRN2", target_bir_lowering=False, num_devices=NC)

    xprop_d = nc.dram_tensor("xprop", [n, bl, CIN], bf16, kind="ExternalInput")
    inpT_d = nc.dram_tensor("inpT", [CIN, nl, b], f16, kind="ExternalInput")
    statT_d = nc.dram_tensor("statT", [DOUT, nl, b], f16, kind="ExternalInput")
    eT_d = nc.dram_tensor("eT", [D, n], f32, kind="ExternalInput")
    eTown_d = nc.dram_tensor("eTown", [D, nl], f32, kind="ExternalInput")
    wgp_d = nc.dram_tensor("wgp", [D, WKg], f16, kind="ExternalInput")
    wup_d = nc.dram_tensor("wup", [D, WKu], f16, kind="ExternalInput")
    bgp_d = nc.dram_tensor("bgp", [D, COg], f16, kind="ExternalInput")
    bup_d = nc.dram_tensor("bup", [D, COu], f16, kind="ExternalInput")
    out_d = nc.dram_tensor("out", [nl, COu, b], f16, kind="ExternalOutput")
    if DBG:
        dbgDinv_d = nc.dram_tensor("dbgDinv", [128, n], f32, kind="ExternalOutput")
        dbgS1r_d = nc.dram_tensor("dbgS1r", [NC, CIN, bl, nl], f16, kind="ExternalOutput")
        dbgZr_d = nc.dram_tensor("dbgZr", [128, nl, b], f16, kind="ExternalOutput")
        dbgW_d = nc.dram_tensor("dbgW", [nl, 1024], f16, kind="ExternalOutput")
        dbgS2r_d = nc.dram_tensor("dbgS2r", [NC, DOUT, bl, nl], f16, kind="ExternalOutput")
        dbgCand_d = nc.dram_tensor("dbgCand", [CIN, nl, b], f16, kind="ExternalOutput")
        dbgZsR_d = nc.dram_tensor("dbgZsR", [NC, nl, bl, DOUT], bf16, kind="ExternalOutput")
        dbgMt4_d = nc.dram_tensor("dbgMt4", [128, MT, NTile], bf16, kind="ExternalOutput")

    with tile.TileContext(nc) as tc:
        with (
            tc.tile_pool(name="persist", bufs=1) as pp,
            tc.tile_pool(name="dram", bufs=1, space="DRAM") as dr,
        ):
            wdram_g = dr.tile([nl, WKg], f16)
            wdram_u = dr.tile([nl, WKu], f16)
            s1_send = dr.tile([NC, CIN, bl, nl], f16)
            s1_recv = dr.tile([NC, CIN, bl, nl], f16)
            zs_send = dr.tile([NC, nl, bl, DOUT], bf16)
            zs_recv = dr.tile([NC, nl, bl, DOUT], bf16)
            s2_send = dr.tile([NC, DOUT, bl, nl], f16)
            s2_recv = dr.tile([NC, DOUT, bl, nl], f16)
            mdram = dr.tile([NT, 128, MT, NTile], bf16)

            eT_sb = pp.tile([128, n], f32r)
            eTown16 = pp.tile([D, nl], f16)
            ones_r = pp.tile([128, 1], bf16)
            dinv_bc = pp.tile([128, n], f32)
            inpT_sb = pp.tile([CIN, nl, b], f16)
            zr16 = pp.tile([128, nl, b], f16)
            bias_g = pp.tile([COg, nl], f32)
            bias_u = pp.tile([128, nl], f32)   # rows 64:128

            def a2a(send, recv):
                if local:
                    nc.sync.dma_start(recv[:], send[:])
                else:
                    nc.gpsimd.collective_compute(
                        "AllToAll", OP.bypass, replica_groups=[list(range(NC))],
                        ins=[send[:].opt()], outs=[recv[:].opt()],
                    )

            for _rep in range(reps):
                # ========== P0: setup + W-compute + bias ==========
                nc.sync.dma_start(inpT_sb[:], inpT_d[:])
                with tc.tile_pool(name="p0tmp", bufs=1) as p0:
                    zscr = p0.tile([128, n], f32)
                    nc.vector.memset(zscr[:], 0.0)
                    nc.vector.tensor_copy(eT_sb[:], zscr[:])
                    nc.sync.dma_start(eT_sb[0:D, :], eT_d[:].bitcast(f32r))
                    eotmp = p0.tile([D, nl], f32)
                    nc.sync.dma_start(eotmp[:], eTown_d[:])
                    nc.vector.tensor_copy(eTown16[:], eotmp[:])
                    nc.vector.memset(ones_r[:], 1.0)

                    bg_sb = p0.tile([D, COg], f16)
                    bu_sb = p0.tile([D, COu], f16)
                    nc.sync.dma_start(bg_sb[:], bgp_d[:])
                    nc.sync.dma_start(bu_sb[:], bup_d[:])
                    with tc.tile_pool(name="psb", bufs=1, space="PSUM") as psb:
                        pb = psb.tile([COg, nl], f32)
                        nc.tensor.matmul(pb[:], bg_sb[:], eTown16[:], start=True, stop=True)
                        nc.vector.tensor_copy(bias_g[:], pb[:])
                        pbu = psb.tile([128, nl], f32, tag="pbu")
                        nc.tensor.matmul(
                            pbu[64:128, :], bu_sb[:], eTown16[:],
                            start=True, stop=True, tile_position=(0, 64),
                        )
                        nc.vector.tensor_copy(bias_u[64:128, :], pbu[64:128, :])

                if upto < 1:
                    continue
                # ========== P1: d1 propagation (+ W-compute filling gaps) ==========
                QW = WKg // 8
                with (
                    tc.tile_pool(name="wpool", bufs=2) as wp,
                    tc.tile_pool(name="wps", bufs=2, space="PSUM") as wps,
                    tc.tile_pool(name="wpsB", bufs=1, space="PSUM") as wpsB,
                    tc.tile_pool(name="wev", bufs=2) as wev,
                ):
                    with (
                        tc.tile_pool(name="xpool", bufs=1) as xp,
                        tc.tile_pool(name="mwork", bufs=1) as mwa,
                        tc.tile_pool(name="mwork2", bufs=2) as mw,
                        tc.tile_pool(name="evw", bufs=2) as evw,
                        tc.tile_pool(name="dsm", bufs=2) as dsm,
                        tc.tile_pool(name="pg", bufs=2, space="PSUM") as pgp,
                        tc.tile_pool(name="pbc", bufs=2, space="PSUM") as pbcp,
                        tc.tile_pool(name="pdp", bufs=1, space="PSUM") as pdp,
                    ):
                        # W-compute chunks, emitted interleaved between P1's
                        # nt iterations so engine-queue order matches
                        # readiness (avoids head-of-line blocking).
                        def emit_w_quarter(wsrc, wdst, h):
                            wg_sb = wp.tile([D, QW], f16, tag="wg_sb", bufs=2)
                            nc.sync.dma_start(wg_sb[:], wsrc[:, h * QW:(h + 1) * QW])
                            for nch in range(nl // 128):
                                for g4 in range(QW // 4096):
                                    wstg = wev.tile([128, 4096], f16, tag="wstg")
                                    for sl in range(8):
                                        # independent PSUM pools per evac
                                        # engine so the DVE and ACT evac
                                        # chains pipeline in parallel
                                        on_act = sl % 4 == 3
                                        if on_act:
                                            pw = wpsB.tile([128, 512], f32, tag="pwB")
                                        else:
                                            pw = wps.tile([128, 512], f32)
                                        nc.tensor.matmul(
                                            pw[:],
                                            eTown16[:, nch * 128:(nch + 1) * 128],
                                            wg_sb[:, g4 * 4096 + sl * 512:
                                                  g4 * 4096 + sl * 512 + 512],
                                            start=True, stop=True,
                                        )
                                        if on_act:
                                            nc.scalar.copy(
                                                wstg[:, sl * 512:(sl + 1) * 512], pw[:]
                                            )
                                        else:
                                            nc.vector.tensor_copy(
                                                wstg[:, sl * 512:(sl + 1) * 512], pw[:]
                                            )
                                    nc.sync.dma_start(
                                        wdst[nch * 128:(nch + 1) * 128,
                                             h * QW + g4 * 4096: h * QW + g4 * 4096 + 4096],
                                        wstg[:],
                                    )

                        wq = ([(wgp_d, wdram_g, h) for h in range(WKg // QW)]
                              + [(wup_d, wdram_u, h) for h in range(WKu // QW)])

                        xprop_sb = xp.tile([128, MT, bl, CIN], bf16)
                        for mc in range(MT):
                            nc.sync.dma_start(
                                xprop_sb[:, mc, :, :],
                                xprop_d[mc * 128:(mc + 1) * 128, :, :],
                            )
                        for nt in range(NT):
                            nsl = slice(nt * NTile, (nt + 1) * NTile)
                            mt2all = mwa.tile([128, MT, NTile], bf16, tag="mt2", bufs=2)
                            pd = pdp.tile([1, NTile], f32)
                            for mc in range(MT):
                                pg = pgp.tile([128, NTile], f32)
                                nc.tensor.matmul(
                                    pg[:], eT_sb[:, mc * 128:(mc + 1) * 128],
                                    eT_sb[:, nsl], start=True, stop=True,
                                )
                                eb = mw.tile([128, NTile], f32, tag="eb")
                                nc.scalar.activation(eb[:], pg[:], AF.Relu)
                                nc.scalar.activation(mt2all[:, mc, :], eb[:], AF.Exp)
                                nc.tensor.matmul(
                                    pd[:], ones_r[:], mt2all[:, mc, :],
                                    start=(mc == 0), stop=(mc == MT - 1),
                                )
                            nc.sync.dma_start(mdram[nt], mt2all[:])
                            dnv = dsm.tile([1, NTile], f32, tag="dnv")
                            nc.vector.reciprocal(dnv[:], pd[:])
                            nc.gpsimd.partition_broadcast(dinv_bc[:, nsl], dnv[:])
                            ev_all = evw.tile([128, BC1, NTile], f16, tag="ev_all")
                            for p in range(BC1):
                                pbc = pbcp.tile([128, NTile], f32, tag="pbc", name=f"pbc{nt}_{p}")
                                for mc in range(MT):
                                    nc.tensor.matmul(
                                        pbc[:], xprop_sb[:, mc, p, :], mt2all[:, mc, :],
                                        start=(mc == 0), stop=(mc == MT - 1),
                                    )
                                nc.vector.tensor_tensor(
                                    ev_all[:, p, :], pbc[:], dinv_bc[:, nsl], OP.mult
                                )
                            for h in range(DPT):
                                nc.sync.dma_start(
                                    s1_send[nt * DPT + h, :, :, :],
                                    ev_all[:, :, h * nl:(h + 1) * nl],
                                )
                            # interleave one W quarter after each nt
                            if nt < len(wq):
                                emit_w_quarter(*wq[nt])
                        # remaining W quarters run in the A2A#1 window
                        for q in wq[NT:]:
                            emit_w_quarter(*q)

                if upto < 2:
                    continue
                a2a(s1_send, s1_recv)
                if DBG:
                    nc.sync.dma_start(dbgS1r_d[:], s1_recv[:])
                    nc.sync.dma_start(dbgDinv_d[:], dinv_bc[:])
                    nc.sync.dma_start(dbgW_d[:], wdram_g[:, 0:1024])

                if upto < 3:
                    continue
                # ========== P3: stage-2 gate ==========
                with (
                    tc.tile_pool(name="ldp", bufs=1) as ldp,
                    tc.tile_pool(name="s2w", bufs=2) as s2w,
                    tc.tile_pool(name="pz", bufs=2, space="PSUM") as pzp,
                ):
                    ld_all = ldp.tile([128, NC, bl, nl], f16)
                    for s in range(NC):
                        nc.sync.dma_start(ld_all[:, s, :, :], s1_recv[s, :, :, :])
                    for t in range(NBat):
                        tsl = slice(t * TB, (t + 1) * TB)
                        xgk1 = s2w.tile([128, TB, b], f16, tag="xgk1")
                        nc.vector.tensor_copy(
                            xgk1[:].rearrange("c n (s b) -> c n s b", s=NC),
                            ld_all[:, :, :, tsl].rearrange("c s b n -> c n s b"),
                        )
                        wg_t = s2w.tile([128, TB, 2, COg], f16, tag="wg_t", bufs=2)
                        nc.sync.dma_start(
                            wg_t[:],
                            wdram_g[tsl, :].rearrange("n (k i o) -> i n k o", k=2, i=CIN),
                        )
                        for g in range(TB // 8):
                            pz = pzp.tile([COg, 8, b], f32)
                            for j8 in range(8):
                                j = g * 8 + j8
                                nc.tensor.matmul(
                                    pz[:, j8, :], wg_t[:, j, 0, :],
                                    inpT_sb[:, t * TB + j, :],
                                    start=(j8 == 0), stop=False, skip_group_check=True,
                                )
                                nc.tensor.matmul(
                                    pz[:, j8, :], wg_t[:, j, 1, :], xgk1[:, j, :],
                                    start=False, stop=(j8 == 7), skip_group_check=True,
                                )
                            zrp = s2w.tile([COg, 8, b], f32, tag="zrp")
                            nc.vector.tensor_tensor(
                                zrp[:], pz[:],
                                bias_g[:, t * TB + g * 8: t * TB + g * 8 + 8, None]
                                .to_broadcast((COg, 8, b)),
                                OP.add,
                            )
                            nc.scalar.activation(
                                zr16[:, t * TB + g * 8: t * TB + g * 8 + 8, :], zrp[:],
                                AF.Sigmoid,
                            )
                        st3 = s2w.tile([DOUT, TB, b], f16, tag="st3", bufs=2)
                        nc.sync.dma_start(st3[:], statT_d[:, tsl, :])
                        zs16 = s2w.tile([DOUT, TB, b], f16, tag="zs16", bufs=2)
                        nc.vector.tensor_tensor(
                            zs16[:], zr16[0:DOUT, tsl, :], st3[:], OP.mult
                        )
                        nc.scalar.copy(inpT_sb[DIN:DIN + 32, tsl, :], zs16[0:32, :, :])
                        nc.scalar.copy(
                            inpT_sb[DIN + 32:DIN + DOUT, tsl, :], zs16[32:64, :, :]
                        )
                        zsbf = s2w.tile([DOUT, TB, b], bf16, tag="zsbf", bufs=2)
                        nc.vector.tensor_tensor(
                            zsbf[:], zr16[0:DOUT, tsl, :], st3[:], OP.mult
                        )
                        # c<->n transpose so the A2A payload is node-major
                        # (receiver loads [m-part, b, c] contiguously).
                        ztile = s2w.tile([TB, b, DOUT], bf16, tag="ztile", bufs=2)
                        for ci in range(2):
                            nc.vector.transpose(
                                ztile[:, :, 32 * ci:32 * ci + 32],
                                zsbf[32 * ci:32 * ci + 32, :, :]
                                .rearrange("c n b -> c b n"),
                            )
                        nc.sync.dma_start(
                            zs_send[:, tsl, :, :].rearrange("d n b c -> n d (b c)"),
                            ztile[:].rearrange("n (d bl) c -> n d (bl c)", d=NC),
                        )

                if upto < 4:
                    continue
                # ========== A2A #2 + d2 propagation ==========
                a2a(zs_send, zs_recv)
                if DBG:
                    nc.sync.dma_start(dbgZr_d[:], zr16[:])
                    nc.sync.dma_start(dbgCand_d[:], inpT_sb[:])
                    nc.sync.dma_start(dbgZsR_d[:], zs_recv[:])
                with (
                    tc.tile_pool(name="zpool", bufs=1) as zp,
                    tc.tile_pool(name="ev4", bufs=2) as ev4w,
                    tc.tile_pool(name="pbc2", bufs=3, space="PSUM") as pbcp2,
                ):
                    zsT_sb = zp.tile([128, MT, bl, DOUT], bf16)
                    if NO_ZST:
                        nc.vector.memset(zsT_sb[:], 0.5)
                    else:
                        for s in range(NC):
                            nc.sync.dma_start(
                                zsT_sb[:, 2 * s:2 * s + 2, :, :],
                                zs_recv[s].rearrange("(h m) b c -> m h b c", h=2),
                            )
                    for nt in range(NT):
                        nsl = slice(nt * NTile, (nt + 1) * NTile)
                        mt2all4 = zp.tile([128, MT, NTile], bf16, tag="mt2all4", bufs=2)
                        if NO_GRE4:
                            nc.vector.memset(mt2all4[:], 0.001)
                        else:
                            nc.sync.dma_start(mt2all4[:], mdram[nt])
                        if NO_BC4:
                            continue
                        for p in range(BC2):
                            pbc = pbcp2.tile([128, NTile], f32, tag="pbc2", name=f"pb2_{nt}_{p}")
                            for mc in range(MT):
                                nc.tensor.matmul(
                                    pbc[:], zsT_sb[:, mc, 2 * p:2 * p + 2, :],
                                    mt2all4[:, mc, :],
                                    start=(mc == 0), stop=(mc == MT - 1),
                                )
                            ev = ev4w.tile([128, NTile], f16, tag="ev")
                            nc.vector.tensor_tensor(
                                ev[:], pbc[:], dinv_bc[:, nsl], OP.mult
                            )
                            for h in range(DPT):
                                nc.sync.dma_start(
                                    s2_send[nt * DPT + h, :, 2 * p, :],
                                    ev[0:DOUT, h * nl:(h + 1) * nl],
                                )
                                nc.sync.dma_start(
                                    s2_send[nt * DPT + h, :, 2 * p + 1, :],
                                    ev[DOUT:128, h * nl:(h + 1) * nl],
                                )
                    if DBG:
                        nc.sync.dma_start(dbgMt4_d[:], mt2all4[:])

                if upto < 5:
                    continue
                # ========== A2A #3 + stage-2 update + GRU ==========
                a2a(s2_send, s2_recv)
                if DBG:
                    nc.sync.dma_start(dbgS2r_d[:], s2_recv[:])
                with (
                    tc.tile_pool(name="ldp5", bufs=1) as ldp5,
                    tc.tile_pool(name="s5w", bufs=2) as s5w,
                    tc.tile_pool(name="ph", bufs=2, space="PSUM") as php,
                ):
                    def load_wu(t):
                        wt = s5w.tile([128, TB, 2, COu], f16, tag="wu_t", bufs=3)
                        nc.sync.dma_start(
                            wt[:],
                            wdram_u[t * TB:(t + 1) * TB, :]
                            .rearrange("n (k i o) -> i n k o", k=2, i=CIN),
                        )
                        return wt

                    # prefetch wu weights ahead of the A2A-gated ld5 loads
                    wu_tiles = [load_wu(t) for t in range(min(3, NBat))]
                    ld5 = ldp5.tile([128, NC, bl, nl], f16)
                    for s in range(NC):
                        nc.sync.dma_start(ld5[0:DIN, s, :, :], s1_recv[s, 0:DIN, :, :])
                        nc.sync.dma_start(ld5[DIN:DIN + DOUT, s, :, :], s2_recv[s, :, :, :])
                        nc.sync.dma_start(
                            ld5[DIN + DOUT:CIN, s, :, :], s1_recv[s, DIN + DOUT:CIN, :, :]
                        )
                    for t in range(NBat):
                        tsl = slice(t * TB, (t + 1) * TB)
                        wu_t = wu_tiles[t]
                        if t + 3 < NBat:
                            wu_tiles.append(load_wu(t + 3))
                        xgk2 = s5w.tile([128, TB, b], f16, tag="xgk2")
                        nc.vector.tensor_copy(
                            xgk2[:].rearrange("c n (s b) -> c n s b", s=NC),
                            ld5[:, :, :, tsl].rearrange("c s b n -> c n s b"),
                        )
                        st5 = s5w.tile([DOUT, TB, b], f16, tag="st5")
                        nc.sync.dma_start(st5[:], statT_d[:, tsl, :])
                        hc_all = s5w.tile([DOUT, TB, b], f16, tag="hc_all")
                        for g in range(TB // 8):
                            gsl8 = slice(g * 8, g * 8 + 8)
                            ph = php.tile([128, 8, b], f32)
                            # all k=0 first: they read only inpT_sb (ready
                            # before A2A#3), so they run in its shadow
                            for j8 in range(8):
                                j = g * 8 + j8
                                nc.tensor.matmul(
                                    ph[64:128, j8, :], wu_t[:, j, 0, :],
                                    inpT_sb[:, t * TB + j, :],
                                    start=(j8 == 0), stop=False, tile_position=(0, 64),
                                    skip_group_check=True,
                                )
                            for j8 in range(8):
                                j = g * 8 + j8
                                nc.tensor.matmul(
                                    ph[64:128, j8, :], wu_t[:, j, 1, :], xgk2[:, j, :],
                                    start=False, stop=(j8 == 7), tile_position=(0, 64),
                                    skip_group_check=True,
                                )
                            hcp = s5w.tile([COu, 8, b], f16, tag="hcp")
                            nc.vector.tensor_tensor(
                                hcp[:], ph[64:128, :, :],
                                bias_u[64:128, t * TB + g * 8: t * TB + g * 8 + 8, None]
                                .to_broadcast((COu, 8, b)),
                                OP.add,
                            )
                            nc.scalar.activation(hc_all[:, gsl8, :], hcp[:], AF.Tanh)
                        # h = hc + r * (state - hc), batched per t in f16 (4x DVE)
                        r16 = s5w.tile([DOUT, TB, b], f16, tag="r16")
                        nc.vector.tensor_copy(r16[:], zr16[64:128, tsl, :])
                        hfin = s5w.tile([DOUT, TB, b], f16, tag="hfin")
                        nc.vector.tensor_tensor(hfin[:], st5[:], hc_all[:], OP.subtract)
                        nc.vector.tensor_tensor(hfin[:], hfin[:], r16[:], OP.mult)
                        nc.vector.tensor_tensor(hfin[:], hfin[:], hc_all[:], OP.add)
                        nc.sync.dma_start(
                            out_d[tsl, :, :].rearrange("n c b -> c n b"),
                            hfin[:],
                        )

    nc.compile()
    return nc


def _get_nc(n, b, reps=1, local=False, upto=9):
    key = (n, b, reps, local, upto)
    if key not in _cache:
        _cache[key] = _build(n, b, reps, local, upto)
    return _cache[key]


def kernel(x, state, emb_dyn, emb_static, Wg, bg, Wu, bu):
    from concourse.bass_utils import run_bass_kernel_spmd

    x = np.asarray(x, np.float32)
    state = np.asarray(state, np.float32)
    emb_dyn = np.asarray(emb_dyn, np.float32)
    emb_static = np.asarray(emb_static, np.float32)
    b, n, _ = x.shape
    bl, nl = b // NC, n // NC

    nc = _get_nc(n, b)

    cat = np.concatenate([x, state, emb_dyn], axis=2)          # [b, n, CIN]
    eT = np.ascontiguousarray(emb_static.T, np.float32)        # [D, n]
    wgp = np.ascontiguousarray(np.asarray(Wg, np.float32).reshape(D, -1)).astype(np.float16)
    wup = np.ascontiguousarray(np.asarray(Wu, np.float32).reshape(D, -1)).astype(np.float16)
    bgp = np.asarray(bg, np.float32).astype(np.float16)
    bup = np.asarray(bu, np.float32).astype(np.float16)

    in_maps = []
    for c in range(NC):
        bsl = slice(c * bl, (c + 1) * bl)
        nsl = slice(c * nl, (c + 1) * nl)
        in_maps.append({
            "xprop": np.ascontiguousarray(cat[bsl].transpose(1, 0, 2)).astype(ml_dtypes.bfloat16),
            "inpT": np.ascontiguousarray(cat[:, nsl].transpose(2, 1, 0)).astype(np.float16),
            "statT": np.ascontiguousarray(state[:, nsl].transpose(2, 1, 0)).astype(np.float16),
            "eT": eT,
            "eTown": np.ascontiguousarray(emb_static[nsl].T, np.float32),
            "wgp": wgp, "wup": wup, "bgp": bgp, "bup": bup,
        })

    global _last_in_maps
    _last_in_maps = in_maps
    res = run_bass_kernel_spmd(nc, in_maps, core_ids=list(range(NC)))

    out = np.empty((b, n, DOUT), np.float32)
    for c in range(NC):
        nsl = slice(c * nl, (c + 1) * nl)
        out[:, nsl, :] = res.results[c]["out"].transpose(2, 0, 1).astype(np.float32)
    return out

